# revision 11
# baseline (speedup 1.0000x reference)
"""Butterfly sparse-attention MLP kernel for 8 Trainium2 NeuronCores.

Computation (from the reference):
    attn = (w1.T @ w2.T) * sparse_mask          # [4096 s, 4096 t]
    y    = gelu(x @ attn + b2)                  # [8, 768, 4096]

sparse_mask is banded: mask[s, t] == 0 whenever |s - t| > 133.  Each core
owns a 512-wide t-block and only needs an 896-wide s-window around it.
Per t-subtile of 128, only 4 of the 7 s-chunks in the window can carry
non-zero attn, so phase B contracts over 512 of s instead of 4096.  Phase A
computes, for s-chunk j, only the t-columns in the true +-133 band hull
(69..394 wide instead of the chunk-aligned 128..512), 23% fewer PE cycles;
the chunk-aligned band regions outside the hull are memset to zero once.

Sharding: tensor-parallel over t (8 blocks of 512).  All per-core variation
is in the input data (windows are zero-padded at the edges; mask zeros make
padded contributions exactly zero), so one SPMD BIR serves all 8 cores.

Schedule: the weight stream (11.5 MB) paces everything early, and the PE
de-ramps its clock whenever it idles, so phase A is emitted in DMA-arrival
order — w1 travels as 7 stripe-major tiles (one [128, 4096] DMA each, 8 KB
descriptors) on the sync queue while w2 streams on the scalar queue, and
the matmul emission follows the (stripe, w2-chunk) arrival fronts.  The
last three stripes' contraction loops are interleaved with the first
n-group's phase-B tiles (which only need earlier stripes), keeping the PE
continuously busy from first weight arrival to the last matmul.  attn PSUM
chunks are packed into 4 banks by hull width so phase A and phase B PSUM
coexist.  x and the mask ride the gpsimd SW-DGE queue as a few big
partition-major DMAs (4 KB descriptors), paced behind the weight stream.
Matmul operands travel as fp16 (accumulation stays fp32 in PSUM).
"""

import numpy as np

B, T, D = 8, 768, 4096
N = B * T            # 6144 rows of x
NCORES = 8
TB = 512             # t-columns per core
P = 128
MARGIN = 192         # s-window extends this far before/after the t-block
SW = TB + 2 * MARGIN  # 896 s-window width
NCH = SW // P        # 7 s-chunks
DCH = D // P         # 32 d-chunks (contraction of phase A)
NQ = TB // P         # 4 t-subtiles per core
GN = 2048            # n-group width in phase B
NG = N // GN         # 3 n-groups
MMN = 512            # moving-operand / PSUM-bank free-dim cap per matmul
BANDCH = 4           # s-chunks feeding one t-subtile (covers +-133 band)
BANDW = 133          # mask support: |s - t| <= BANDW
W2PACK = 4           # w2T d-chunks packed per DMA row (4 KB descriptors)
XSPLIT = 4           # leading s-chunks per x group DMA (rest in 2nd DMA)

# attn PSUM bank per chunk: a bank hosts one OPEN accumulation chain at a
# time, so the four concurrent pass-1 chains (j=0..3) get their own banks
# and the sequential pass-2 chains (j=4..6) reuse banks whose chain closed
PBANK = {0: 0, 1: 1, 2: 2, 3: 3, 4: 0, 5: 1, 6: 2}

_NC = None


def _band(j):
    """Chunk-aligned t-column range [lo, hi) of attn chunk j phase B reads."""
    lo = P * max(0, j - (BANDCH - 1))
    hi = P * min(NQ - 1, j) + P
    return lo, hi


def _hull(j):
    """True mask-support t-range [lo, hi) of attn chunk j (|s-t| <= 133)."""
    lo = max(0, P * j - MARGIN - BANDW)        # 128j - 325
    hi = min(TB, P * j + P - MARGIN + BANDW)   # 128j + 69
    return lo, hi


def _build_module():
    from concourse import bacc, bass, mybir, tile
    from concourse.tile_rust import add_dep_helper

    f32 = mybir.dt.float32
    f16 = mybir.dt.float16
    PSUM = bass.MemorySpace.PSUM

    nc = bacc.Bacc("TRN2", target_bir_lowering=False, debug=False)
    xT_d = nc.declare_dram_parameter("xT_s", [P, NCH, N], f16, isOutput=False)
    w1_d = nc.declare_dram_parameter(
        "w1_s", [NCH, P, DCH * P], f16, isOutput=False)
    w2T_d = nc.declare_dram_parameter(
        "w2T_s", [DCH // W2PACK, P, W2PACK * TB], f16, isOutput=False)
    mask_d = nc.declare_dram_parameter(
        "mask_s", [P, NCH * TB], f16, isOutput=False)
    b2_d = nc.declare_dram_parameter("b2c_s", [P, NQ], f32, isOutput=False)
    yT_d = nc.declare_dram_parameter("yT_s", [TB, N], f16, isOutput=True)

    with tile.TileContext(nc) as tc:
        with (
            tc.tile_pool(name="const", bufs=1) as cpool,
            tc.tile_pool(name="attn", bufs=1) as apool,
            tc.tile_pool(name="mp", bufs=1) as mp,
            tc.tile_pool(name="w1p", bufs=1) as w1p,
            tc.tile_pool(name="w2p", bufs=1) as w2p,
            tc.tile_pool(name="xp", bufs=2) as xp,
            tc.tile_pool(name="yp", bufs=4) as yp,
            tc.tile_pool(name="psA", bufs=1, space=PSUM) as psA,
            tc.tile_pool(name="psB", bufs=2, space=PSUM) as psB,
        ):
            b2_t = cpool.tile([P, NQ], f32)
            nc.gpsimd.dma_start(b2_t[:], b2_d[:])
            m_t = mp.tile([P, NCH * TB], f16, name="m_t")
            nc.gpsimd.dma_start(m_t[:], mask_d[:])

            # attn SBUF tiles: zero the band-minus-hull gap regions once so
            # phase B reads exact zeros there (mask support ends at the hull)
            attn_sb = []
            for j in range(NCH):
                a_t = apool.tile([P, TB], f16, name=f"attn_sb{j}")
                blo, bhi = _band(j)
                hlo, hhi = _hull(j)
                if blo < hlo:
                    nc.vector.memset(a_t[:, blo:hlo], 0)
                if hhi < bhi:
                    nc.vector.memset(a_t[:, hhi:bhi], 0)
                attn_sb.append(a_t)

            pbanks = [psA.tile([P, MMN], f32, name=f"pb{b}") for b in range(4)]

            def attn_ps(j):
                lo, hi = _hull(j)
                return pbanks[PBANK[j]][:, :hi - lo]

            # ---- weight DMAs ----
            # w1 stripes (1 MB, 8 KB descriptors) on sync in pass order;
            # the first is split so the very first matmul starts sooner.
            w1_t = [w1p.tile([P, DCH * P], f16, name=f"w1s{j}")
                    for j in range(NCH)]
            w1_i = {}
            for j in (3, 2, 1, 0, 4, 5, 6):
                if j == 3:
                    nc.sync.dma_start(w1_t[3][:, :2 * P], w1_d[3][:, :2 * P])
                    nc.sync.dma_start(w1_t[3][:, 2 * P:8 * P],
                                      w1_d[3][:, 2 * P:8 * P])
                    w1_i[3] = nc.sync.dma_start(
                        w1_t[3][:, 8 * P:], w1_d[3][:, 8 * P:])
                else:
                    w1_i[j] = nc.sync.dma_start(w1_t[j][:], w1_d[j])
            # w2 chunks (1 MB, 4 KB descriptors) on scalar; first split too
            w2_t = [w2p.tile([P, W2PACK * TB], f16, name=f"w2c{bb}")
                    for bb in range(DCH // W2PACK)]
            w2_i = []
            for bb in range(DCH // W2PACK):
                if bb == 0:
                    nc.scalar.dma_start(w2_t[0][:, :TB], w2T_d[0][:, :TB])
                    w2_i.append(nc.scalar.dma_start(
                        w2_t[0][:, TB:], w2T_d[0][:, TB:]))
                else:
                    w2_i.append(nc.scalar.dma_start(w2_t[bb][:], w2T_d[bb]))

            def w2sl(k, lo, hi):
                bb, sl = k // W2PACK, k % W2PACK
                return w2_t[bb][:, sl * TB + lo:sl * TB + hi]

            def mm(j, k):
                lo, hi = _hull(j)
                nc.tensor.matmul(
                    attn_ps(j),
                    w1_t[j][:, k * P:(k + 1) * P],
                    w2sl(k, lo, hi),
                    start=(k == 0),
                    stop=(k == DCH - 1),
                )

            def maskmul(j):
                lo, hi = _hull(j)
                nc.vector.tensor_mul(
                    attn_sb[j][:, lo:hi], attn_ps(j),
                    m_t[:, j * TB + lo:j * TB + hi])

            # ---- Phase A pass 1 (stripes 3,2,1,0) in DMA-arrival order ----
            order = []
            order += [(3, k) for k in range(0, 8)]
            order += [(2, k) for k in range(0, 8)]
            order += [(3, k) for k in range(8, 12)]
            order += [(2, k) for k in range(8, 12)]
            order += [(1, k) for k in range(0, 12)]
            order += [(j, k) for k in range(12, 16) for j in (3, 2, 1)]
            order += [(0, k) for k in range(0, 16)]
            order += [(j, k) for k in range(16, DCH) for j in (3, 2, 1, 0)]
            for j, k in order:
                mm(j, k)
            for j in (0, 1, 2, 3):
                maskmul(j)

            # ---- x prefetch: partition-major DMAs paced behind weights ----
            # group 0 lands in four ~1 MB slices gated late in the weight
            # stream (so weights keep bandwidth priority but the slices the
            # first phase-B tiles need are there when attn completes);
            # groups 1-2 follow the last stripes in SW-DGE queue order.
            x_t = []
            xparts = {0: ((0, 2), (2, 4), (4, 6), (6, 7)),
                      1: ((0, XSPLIT), (XSPLIT, NCH)),
                      2: ((0, XSPLIT), (XSPLIT, NCH))}
            xgates = {(0, 0): w2_i[5], (0, 1): w2_i[7],
                      (0, 2): w1_i[0], (0, 3): w1_i[4],
                      (1, 0): w1_i[5], (1, 1): w1_i[5],
                      (2, 0): w1_i[6], (2, 1): w1_i[6]}
            for g in range(NG):
                xt = xp.tile([P, NCH * GN], f16, name="x_t", tag="x_t")
                for part, (jlo, jhi) in enumerate(xparts[g]):
                    xi = nc.gpsimd.dma_start(
                        xt[:, jlo * GN:jhi * GN],
                        xT_d[:, jlo:jhi, g * GN:(g + 1) * GN],
                    )
                    add_dep_helper(
                        xi.ins, xgates[(g, part)].ins,
                        sync=True, reason="pace x prefetch behind weights",
                    )
                x_t.append(xt)

            # ---- Phase B tiles, interleaved with phase A pass 2 ----
            def emit_b(g, q, last=False):
                xt = x_t[g]
                y_sb = yp.tile([P, GN], f16, name="y_sb", tag="y_sb")
                for h in range(GN // (2 * MMN)):
                    y_ps = psB.tile([P, 2 * MMN], f32, name="y_ps", tag="y_ps")
                    for hh in range(2):
                        osl = slice(hh * MMN, (hh + 1) * MMN)
                        nlo = (2 * h + hh) * MMN
                        for c in range(BANDCH):
                            j = q + c
                            nc.tensor.matmul(
                                y_ps[:, osl],
                                attn_sb[j][:, q * P:(q + 1) * P],
                                xt[:, j * GN + nlo:j * GN + nlo + MMN],
                                start=(c == 0),
                                stop=(c == BANDCH - 1),
                            )
                    if last and h == GN // (2 * MMN) - 1:
                        # split the final gelu so the tail after the last
                        # matmul is as short as possible
                        for hh in range(2):
                            nc.scalar.activation(
                                y_sb[:, (2 * h + hh) * MMN:
                                     (2 * h + hh + 1) * MMN],
                                y_ps[:, hh * MMN:(hh + 1) * MMN],
                                mybir.ActivationFunctionType.Gelu,
                                bias=b2_t[:, q:q + 1],
                                scale=1.0,
                            )
                    else:
                        nc.scalar.activation(
                            y_sb[:, 2 * h * MMN:2 * (h + 1) * MMN],
                            y_ps[:],
                            mybir.ActivationFunctionType.Gelu,
                            bias=b2_t[:, q:q + 1],
                            scale=1.0,
                        )
                if last:
                    # store the two halves in parallel on both queues
                    nc.scalar.dma_start(
                        yT_d[q * P:(q + 1) * P, g * GN:g * GN + GN // 2],
                        y_sb[:, :GN // 2])
                    nc.sync.dma_start(
                        yT_d[q * P:(q + 1) * P, g * GN + GN // 2:(g + 1) * GN],
                        y_sb[:, GN // 2:])
                else:
                    st_eng = nc.scalar if (g + q) % 2 == 0 else nc.sync
                    st_eng.dma_start(
                        yT_d[q * P:(q + 1) * P, g * GN:(g + 1) * GN],
                        y_sb[:])

            emit_b(0, 0)
            for j in (4, 5, 6):
                for k in range(DCH):
                    mm(j, k)
                maskmul(j)
                emit_b(0, j - 3)
            for g in range(1, NG):
                for q in range(NQ):
                    emit_b(g, q, last=(g == NG - 1 and q == NQ - 1))

    nc.compile()
    nc.finalize()
    return nc


def _get_nc():
    global _NC
    if _NC is None:
        _NC = _build_module()
    return _NC


def prepare_in_maps(x, w1, w2, b2, sparse_mask):
    x = np.asarray(x, dtype=np.float32)
    w1 = np.asarray(w1, dtype=np.float32)
    w2 = np.asarray(w2, dtype=np.float32)
    b2 = np.asarray(b2, dtype=np.float32)
    sparse_mask = np.asarray(sparse_mask, dtype=np.float32)

    xT = np.ascontiguousarray(x.reshape(N, D).T.astype(np.float16))   # [s, n]
    w2T = np.ascontiguousarray(w2.T.astype(np.float16))               # [d, t]

    # Zero-pad the s axis by MARGIN on both sides so every core's window is
    # a plain slice; mask zeros make the padded rows contribute nothing.
    xT_pad = np.zeros((D + 2 * MARGIN, N), dtype=np.float16)
    xT_pad[MARGIN:MARGIN + D] = xT
    w1_pad = np.zeros((D, D + 2 * MARGIN), dtype=np.float16)
    w1_pad[:, MARGIN:MARGIN + D] = w1.astype(np.float16)
    mask_pad = np.zeros((D + 2 * MARGIN, D), dtype=np.float16)
    mask_pad[MARGIN:MARGIN + D] = sparse_mask.astype(np.float16)

    in_maps = []
    for i in range(NCORES):
        s0 = i * TB           # window start in padded coords
        t0 = i * TB
        w1win = w1_pad[:, s0:s0 + SW]                     # [D, SW]
        # stripe-major: [NCH, P, DCH*P], w1_s[j, p, k*P+u] = w1win[k*P+p,
        # j*P+u]; one 8 KB-descriptor DMA per stripe
        w1_s = (w1win.reshape(DCH, P, NCH, P)
                .transpose(2, 1, 0, 3)
                .reshape(NCH, P, DCH * P))
        w2win = w2T[:, t0:t0 + TB]                        # [D, TB]
        w2_s = (w2win.reshape(DCH // W2PACK, W2PACK, P, TB)
                .transpose(0, 2, 1, 3)
                .reshape(DCH // W2PACK, P, W2PACK * TB))
        # x window and mask, partition-major: one DMA covers several
        # s-chunks with >= 4 KB descriptors
        xwin = (xT_pad[s0:s0 + SW].reshape(NCH, P, N)
                .transpose(1, 0, 2))                      # [P, NCH, N]
        mwin = (mask_pad[s0:s0 + SW, t0:t0 + TB].reshape(NCH, P, TB)
                .transpose(1, 0, 2)
                .reshape(P, NCH * TB))                    # [P, NCH*TB]
        in_maps.append({
            "xT_s": np.ascontiguousarray(xwin),
            "w1_s": np.ascontiguousarray(w1_s),
            "w2T_s": np.ascontiguousarray(w2_s),
            "mask_s": np.ascontiguousarray(mwin),
            "b2c_s": np.ascontiguousarray(b2[t0:t0 + TB].reshape(NQ, P).T),
        })
    return in_maps


def assemble(results):
    out = np.empty((N, D), dtype=np.float32)
    for i in range(NCORES):
        out[:, i * TB:(i + 1) * TB] = results[i]["yT_s"].T.astype(np.float32)
    return out.reshape(B, T, D)


def _band_ok(sparse_mask):
    """The Bass kernel only computes attn where each core's hull window
    covers the mask; verify every mask nonzero falls inside that region."""
    s_idx, t_idx = np.nonzero(np.asarray(sparse_mask) != 0)
    if len(s_idx) == 0:
        return True
    if np.abs(s_idx.astype(np.int64) - t_idx).max() > BANDW:
        return False
    w0 = (t_idx // TB) * TB - MARGIN          # per-core s-window start
    j = (s_idx - w0) // P                     # s-chunk within window
    q = (t_idx % TB) // P                     # t-subtile
    return bool(np.all((j >= q) & (j <= q + BANDCH - 1)
                       & (s_idx >= w0) & (s_idx < w0 + SW)))


def _reference_fallback(x, w1, w2, b2, sparse_mask):
    import jax
    import jax.numpy as jnp

    cpu = jax.devices("cpu")[0]
    with jax.default_device(cpu):
        attn = jnp.einsum("ds,td->st", jnp.asarray(w1), jnp.asarray(w2))
        attn = attn * jnp.asarray(sparse_mask)
        y = jnp.einsum("bds,st->bdt", jnp.asarray(x), attn) + jnp.asarray(b2)
        return np.asarray(jax.nn.gelu(y, approximate=False), dtype=np.float32)


def kernel(x, w1, w2, b2, sparse_mask):
    import time

    from concourse.bass_utils import run_bass_kernel_spmd

    if (np.shape(x) != (B, T, D) or np.shape(w1) != (D, D)
            or np.shape(w2) != (D, D) or np.shape(b2) != (D,)
            or np.shape(sparse_mask) != (D, D) or not _band_ok(sparse_mask)):
        return _reference_fallback(x, w1, w2, b2, sparse_mask)

    in_maps = prepare_in_maps(x, w1, w2, b2, sparse_mask)
    nc = _get_nc()
    last_err = None
    for attempt in range(3):
        try:
            res = run_bass_kernel_spmd(nc, in_maps, list(range(NCORES)))
            return assemble(res.results)
        except Exception as e:  # transient NRT/device errors: retry
            last_err = e
            time.sleep(2.0 * (attempt + 1))
    raise last_err


# revision 12
# speedup vs baseline: 1.0138x; 1.0138x over previous
"""Butterfly sparse-attention MLP kernel for 8 Trainium2 NeuronCores.

Computation (from the reference):
    attn = (w1.T @ w2.T) * sparse_mask          # [4096 s, 4096 t]
    y    = gelu(x @ attn + b2)                  # [8, 768, 4096]

sparse_mask is banded: mask[s, t] == 0 whenever |s - t| > 133.  Each core
owns a 512-wide t-block and only needs an 896-wide s-window around it.
Per t-subtile of 128, only 4 of the 7 s-chunks in the window can carry
non-zero attn, so phase B contracts over 512 of s instead of 4096.  Phase A
computes, for s-chunk j, only the t-columns in the true +-133 band hull
(69..394 wide instead of the chunk-aligned 128..512), 23% fewer PE cycles;
the chunk-aligned band regions outside the hull are memset to zero once.

Sharding: tensor-parallel over t (8 blocks of 512).  All per-core variation
is in the input data (windows are zero-padded at the edges; mask zeros make
padded contributions exactly zero), so one SPMD BIR serves all 8 cores.

Matmul operands travel as fp16 (accumulation stays fp32 in PSUM), halving
HBM traffic.  Each HW-DGE queue generates ~60M descriptors/s, so every
tensor is host-shuffled to 3.5-4 KB descriptors and the 24 weight DMAs
alternate between the sync and scalar queues (1536 descriptors each) —
the weight stream, not descriptor generation, then paces phase A.  x and
the mask ride the gpsimd SW-DGE queue as a few big partition-major DMAs;
x is triple-buffered and paced behind the w1 stream so weights keep full
bandwidth until attn is done but the first n-group lands right as phase B
can start.
"""

import numpy as np

B, T, D = 8, 768, 4096
N = B * T            # 6144 rows of x
NCORES = 8
TB = 512             # t-columns per core
P = 128
MARGIN = 192         # s-window extends this far before/after the t-block
SW = TB + 2 * MARGIN  # 896 s-window width
NCH = SW // P        # 7 s-chunks
DCH = D // P         # 32 d-chunks (contraction of phase A)
NQ = TB // P         # 4 t-subtiles per core
GN = 2048            # n-group width in phase B
NG = N // GN         # 3 n-groups
MMN = 512            # moving-operand / PSUM-bank free-dim cap per matmul
BANDCH = 4           # s-chunks feeding one t-subtile (covers +-133 band)
BANDW = 133          # mask support: |s - t| <= BANDW
W1PACK = 2           # w1 d-chunks packed per DMA row (3.5 KB descriptors)
W2PACK = 4           # w2T d-chunks packed per DMA row (4 KB descriptors)
XSPLIT = 4           # leading s-chunks per x group DMA (rest in 2nd DMA)

_NC = None


def _band(j):
    """Chunk-aligned t-column range [lo, hi) of attn chunk j phase B reads."""
    lo = P * max(0, j - (BANDCH - 1))
    hi = P * min(NQ - 1, j) + P
    return lo, hi


def _hull(j):
    """True mask-support t-range [lo, hi) of attn chunk j (|s-t| <= 133)."""
    lo = max(0, P * j - MARGIN - BANDW)        # 128j - 325
    hi = min(TB, P * j + P - MARGIN + BANDW)   # 128j + 69
    return lo, hi


def _build_module():
    from concourse import bacc, bass, mybir, tile
    from concourse.tile_rust import add_dep_helper

    f32 = mybir.dt.float32
    f16 = mybir.dt.float16
    PSUM = bass.MemorySpace.PSUM

    nc = bacc.Bacc("TRN2", target_bir_lowering=False, debug=False)
    xT_d = nc.declare_dram_parameter("xT_s", [P, NCH, N], f16, isOutput=False)
    w1_d = nc.declare_dram_parameter(
        "w1_s", [DCH // W1PACK, P, W1PACK * SW], f16, isOutput=False)
    w2T_d = nc.declare_dram_parameter(
        "w2T_s", [DCH // W2PACK, P, W2PACK * TB], f16, isOutput=False)
    mask_d = nc.declare_dram_parameter(
        "mask_s", [P, NCH * TB], f16, isOutput=False)
    b2_d = nc.declare_dram_parameter("b2c_s", [P, NQ], f32, isOutput=False)
    yT_d = nc.declare_dram_parameter("yT_s", [TB, N], f16, isOutput=True)

    with tile.TileContext(nc) as tc:
        with (
            tc.tile_pool(name="const", bufs=1) as cpool,
            tc.tile_pool(name="attn", bufs=1) as apool,
            tc.tile_pool(name="mp", bufs=1) as mp,
            tc.tile_pool(name="xp", bufs=NG) as xp,
            tc.tile_pool(name="yp", bufs=4) as yp,
        ):
            b2_t = cpool.tile([P, NQ], f32)
            nc.gpsimd.dma_start(b2_t[:], b2_d[:])
            m_t = mp.tile([P, NCH * TB], f16, name="m_t")
            nc.gpsimd.dma_start(m_t[:], mask_d[:])

            # attn SBUF tiles: zero the band-minus-hull gap regions once so
            # phase B reads exact zeros there (mask support ends at the hull)
            attn_sb = []
            for j in range(NCH):
                a_t = apool.tile([P, TB], f16, name=f"attn_sb{j}")
                blo, bhi = _band(j)
                hlo, hhi = _hull(j)
                if blo < hlo:
                    nc.vector.memset(a_t[:, blo:hlo], 0)
                if hhi < bhi:
                    nc.vector.memset(a_t[:, hhi:bhi], 0)
                attn_sb.append(a_t)

            # ---- Phase A: attn[s, t] = (w1.T @ w2T) * mask on the hull ----
            # weight DMAs alternate sync/scalar so each queue generates only
            # half the descriptors; arrival order tracks k-consumption order
            w_eng = [nc.sync, nc.scalar]
            w_idx = 0

            def w_dma(dst, src):
                nonlocal w_idx
                eng = w_eng[w_idx % 2]
                w_idx += 1
                return eng.dma_start(dst, src)

            w1_insts = []
            with (
                tc.tile_pool(name="w1p", bufs=8) as w1p,
                tc.tile_pool(name="w2p", bufs=4) as w2p,
                tc.tile_pool(name="psA", bufs=1, space=PSUM) as psA,
            ):
                attn_ps = [
                    psA.tile([P, TB], f32, name=f"attn_ps{j}") for j in range(NCH)
                ]
                for bb in range(DCH // W2PACK):
                    w2_t = w2p.tile([P, W2PACK * TB], f16)
                    if bb == 0:
                        # split so the k=0 matmuls can start sooner
                        nc.sync.dma_start(w2_t[:, :TB], w2T_d[0][:, :TB])
                        w_dma(w2_t[:, TB:], w2T_d[0][:, TB:])
                    else:
                        w_dma(w2_t[:], w2T_d[bb])
                    for hb in range(W2PACK // W1PACK):
                        pi = bb * (W2PACK // W1PACK) + hb
                        w1_t = w1p.tile([P, W1PACK * SW], f16)
                        if pi == 0:
                            nc.scalar.dma_start(w1_t[:, :SW], w1_d[0][:, :SW])
                            w1_insts.append(w_dma(w1_t[:, SW:], w1_d[0][:, SW:]))
                        else:
                            w1_insts.append(w_dma(w1_t[:], w1_d[pi]))
                        for half in range(W1PACK):
                            k = bb * W2PACK + hb * W1PACK + half
                            w1sl = w1_t[:, half * SW:(half + 1) * SW]
                            w2sl = w2_t[:, (hb * W1PACK + half) * TB:
                                        (hb * W1PACK + half + 1) * TB]
                            for j in (3, 2, 4, 1, 5, 0, 6):
                                lo, hi = _hull(j)
                                nc.tensor.matmul(
                                    attn_ps[j][:, lo:hi],
                                    w1sl[:, j * P:(j + 1) * P],
                                    w2sl[:, lo:hi],
                                    start=(k == 0),
                                    stop=(k == DCH - 1),
                                )
                for j in range(NCH):
                    lo, hi = _hull(j)
                    nc.vector.tensor_mul(
                        attn_sb[j][:, lo:hi], attn_ps[j][:, lo:hi],
                        m_t[:, j * TB + lo:j * TB + hi]
                    )

            # ---- Phase B: yT[t, n] = gelu(attn.T @ xT + b2) on the band ----
            with tc.tile_pool(name="psB", bufs=4, space=PSUM) as psB:
                x_t = []
                xgates = {(0, 0): 10, (0, 1): 12, (1, 0): 14, (1, 1): 15,
                          (2, 0): 15, (2, 1): 15}
                for g in range(NG):
                    xt = xp.tile([P, NCH * GN], f16, name="x_t", tag="x_t")
                    for part, (jlo, jhi) in enumerate(((0, XSPLIT), (XSPLIT, NCH))):
                        xi = nc.gpsimd.dma_start(
                            xt[:, jlo * GN:jhi * GN],
                            xT_d[:, jlo:jhi, g * GN:(g + 1) * GN],
                        )
                        add_dep_helper(
                            xi.ins, w1_insts[xgates[(g, part)]].ins,
                            sync=True, reason="pace x prefetch behind w1",
                        )
                    x_t.append(xt)
                for g in range(NG):
                    xt = x_t[g]
                    for q in range(NQ):
                        y_sb = yp.tile([P, GN], f16, name="y_sb", tag="y_sb")
                        last = (g == NG - 1 and q == NQ - 1)
                        for h in range(GN // (2 * MMN)):
                            y_ps = psB.tile([P, 2 * MMN], f32, name="y_ps",
                                            tag="y_ps")
                            for hh in range(2):
                                osl = slice(hh * MMN, (hh + 1) * MMN)
                                nlo = (2 * h + hh) * MMN
                                for c in range(BANDCH):
                                    j = q + c
                                    nc.tensor.matmul(
                                        y_ps[:, osl],
                                        attn_sb[j][:, q * P:(q + 1) * P],
                                        xt[:, j * GN + nlo:j * GN + nlo + MMN],
                                        start=(c == 0),
                                        stop=(c == BANDCH - 1),
                                    )
                            if last and h == GN // (2 * MMN) - 1:
                                # split the final gelu so the tail after the
                                # last matmul is as short as possible
                                for hh in range(2):
                                    nc.scalar.activation(
                                        y_sb[:, (2 * h + hh) * MMN:
                                             (2 * h + hh + 1) * MMN],
                                        y_ps[:, hh * MMN:(hh + 1) * MMN],
                                        mybir.ActivationFunctionType.Gelu,
                                        bias=b2_t[:, q:q + 1],
                                        scale=1.0,
                                    )
                            else:
                                nc.scalar.activation(
                                    y_sb[:, 2 * h * MMN:2 * (h + 1) * MMN],
                                    y_ps[:],
                                    mybir.ActivationFunctionType.Gelu,
                                    bias=b2_t[:, q:q + 1],
                                    scale=1.0,
                                )
                        if last:
                            # store the two halves in parallel on both queues
                            nc.scalar.dma_start(
                                yT_d[q * P:(q + 1) * P,
                                     g * GN:g * GN + GN // 2],
                                y_sb[:, :GN // 2])
                            nc.sync.dma_start(
                                yT_d[q * P:(q + 1) * P,
                                     g * GN + GN // 2:(g + 1) * GN],
                                y_sb[:, GN // 2:])
                        else:
                            st_eng = nc.scalar if (g + q) % 2 == 0 else nc.sync
                            st_eng.dma_start(
                                yT_d[q * P:(q + 1) * P, g * GN:(g + 1) * GN],
                                y_sb[:])

    nc.compile()
    nc.finalize()
    return nc


def _get_nc():
    global _NC
    if _NC is None:
        _NC = _build_module()
    return _NC


def prepare_in_maps(x, w1, w2, b2, sparse_mask):
    x = np.asarray(x, dtype=np.float32)
    w1 = np.asarray(w1, dtype=np.float32)
    w2 = np.asarray(w2, dtype=np.float32)
    b2 = np.asarray(b2, dtype=np.float32)
    sparse_mask = np.asarray(sparse_mask, dtype=np.float32)

    xT = np.ascontiguousarray(x.reshape(N, D).T.astype(np.float16))   # [s, n]
    w2T = np.ascontiguousarray(w2.T.astype(np.float16))               # [d, t]

    # Zero-pad the s axis by MARGIN on both sides so every core's window is
    # a plain slice; mask zeros make the padded rows contribute nothing.
    xT_pad = np.zeros((D + 2 * MARGIN, N), dtype=np.float16)
    xT_pad[MARGIN:MARGIN + D] = xT
    w1_pad = np.zeros((D, D + 2 * MARGIN), dtype=np.float16)
    w1_pad[:, MARGIN:MARGIN + D] = w1.astype(np.float16)
    mask_pad = np.zeros((D + 2 * MARGIN, D), dtype=np.float16)
    mask_pad[MARGIN:MARGIN + D] = sparse_mask.astype(np.float16)

    in_maps = []
    for i in range(NCORES):
        s0 = i * TB           # window start in padded coords
        t0 = i * TB
        w1win = w1_pad[:, s0:s0 + SW]                     # [D, SW]
        # pack W1PACK d-chunks per DMA row: [DCH/W1PACK, P, W1PACK*SW]
        w1_s = (w1win.reshape(DCH // W1PACK, W1PACK, P, SW)
                .transpose(0, 2, 1, 3)
                .reshape(DCH // W1PACK, P, W1PACK * SW))
        w2win = w2T[:, t0:t0 + TB]                        # [D, TB]
        w2_s = (w2win.reshape(DCH // W2PACK, W2PACK, P, TB)
                .transpose(0, 2, 1, 3)
                .reshape(DCH // W2PACK, P, W2PACK * TB))
        # x window and mask, partition-major: one DMA covers several
        # s-chunks with >= 4 KB descriptors
        xwin = (xT_pad[s0:s0 + SW].reshape(NCH, P, N)
                .transpose(1, 0, 2))                      # [P, NCH, N]
        mwin = (mask_pad[s0:s0 + SW, t0:t0 + TB].reshape(NCH, P, TB)
                .transpose(1, 0, 2)
                .reshape(P, NCH * TB))                    # [P, NCH*TB]
        in_maps.append({
            "xT_s": np.ascontiguousarray(xwin),
            "w1_s": np.ascontiguousarray(w1_s),
            "w2T_s": np.ascontiguousarray(w2_s),
            "mask_s": np.ascontiguousarray(mwin),
            "b2c_s": np.ascontiguousarray(b2[t0:t0 + TB].reshape(NQ, P).T),
        })
    return in_maps


def assemble(results):
    out = np.empty((N, D), dtype=np.float32)
    for i in range(NCORES):
        out[:, i * TB:(i + 1) * TB] = results[i]["yT_s"].T.astype(np.float32)
    return out.reshape(B, T, D)


def _band_ok(sparse_mask):
    """The Bass kernel only computes attn where each core's hull window
    covers the mask; verify every mask nonzero falls inside that region."""
    s_idx, t_idx = np.nonzero(np.asarray(sparse_mask) != 0)
    if len(s_idx) == 0:
        return True
    if np.abs(s_idx.astype(np.int64) - t_idx).max() > BANDW:
        return False
    w0 = (t_idx // TB) * TB - MARGIN          # per-core s-window start
    j = (s_idx - w0) // P                     # s-chunk within window
    q = (t_idx % TB) // P                     # t-subtile
    return bool(np.all((j >= q) & (j <= q + BANDCH - 1)
                       & (s_idx >= w0) & (s_idx < w0 + SW)))


def _reference_fallback(x, w1, w2, b2, sparse_mask):
    import jax
    import jax.numpy as jnp

    cpu = jax.devices("cpu")[0]
    with jax.default_device(cpu):
        attn = jnp.einsum("ds,td->st", jnp.asarray(w1), jnp.asarray(w2))
        attn = attn * jnp.asarray(sparse_mask)
        y = jnp.einsum("bds,st->bdt", jnp.asarray(x), attn) + jnp.asarray(b2)
        return np.asarray(jax.nn.gelu(y, approximate=False), dtype=np.float32)


def kernel(x, w1, w2, b2, sparse_mask):
    import time

    from concourse.bass_utils import run_bass_kernel_spmd

    if (np.shape(x) != (B, T, D) or np.shape(w1) != (D, D)
            or np.shape(w2) != (D, D) or np.shape(b2) != (D,)
            or np.shape(sparse_mask) != (D, D) or not _band_ok(sparse_mask)):
        return _reference_fallback(x, w1, w2, b2, sparse_mask)

    in_maps = prepare_in_maps(x, w1, w2, b2, sparse_mask)
    nc = _get_nc()
    last_err = None
    for attempt in range(3):
        try:
            res = run_bass_kernel_spmd(nc, in_maps, list(range(NCORES)))
            return assemble(res.results)
        except Exception as e:  # transient NRT/device errors: retry
            last_err = e
            time.sleep(2.0 * (attempt + 1))
    raise last_err


# revision 15
# speedup vs baseline: 1.0545x; 1.0401x over previous
"""Butterfly sparse-attention MLP kernel for 8 Trainium2 NeuronCores.

Computation (from the reference):
    attn = (w1.T @ w2.T) * sparse_mask          # [4096 s, 4096 t]
    y    = gelu(x @ attn + b2)                  # [8, 768, 4096]

sparse_mask is banded: mask[s, t] == 0 whenever |s - t| > 133.  Each core
owns a 512-wide t-block and only needs an 896-wide s-window around it.
Per t-subtile of 128, only 4 of the 7 s-chunks in the window can carry
non-zero attn, so phase B contracts over 512 of s instead of 4096.  Phase A
computes, for s-chunk j, only the t-columns in the true +-133 band hull
(69..394 wide instead of the chunk-aligned 128..512), 23% fewer PE cycles;
the chunk-aligned band regions outside the hull are memset to zero once.

Sharding: tensor-parallel over t (8 blocks of 512).  All per-core variation
is in the input data (windows are zero-padded at the edges; mask zeros make
padded contributions exactly zero), so one SPMD BIR serves all 8 cores.

Matmul operands travel as fp16 (accumulation stays fp32 in PSUM), halving
HBM traffic.  Each HW-DGE queue generates ~60M descriptors/s, so every
tensor is host-shuffled to 3.5-4 KB descriptors and the 24 weight DMAs
alternate between the sync and scalar queues (1536 descriptors each) —
the weight stream, not descriptor generation, then paces phase A.  x and
the mask ride the gpsimd SW-DGE queue as a few big partition-major DMAs;
x is triple-buffered and paced behind the w1 stream so weights keep full
bandwidth until attn is done but the first n-group lands right as phase B
can start.
"""

import numpy as np

B, T, D = 8, 768, 4096
N = B * T            # 6144 rows of x
NCORES = 8
TB = 512             # t-columns per core
P = 128
MARGIN = 192         # s-window extends this far before/after the t-block
SW = TB + 2 * MARGIN  # 896 s-window width
NCH = SW // P        # 7 s-chunks
DCH = D // P         # 32 d-chunks (contraction of phase A)
NQ = TB // P         # 4 t-subtiles per core
GN = 2048            # n-group width in phase B
NG = N // GN         # 3 n-groups
MMN = 512            # moving-operand / PSUM-bank free-dim cap per matmul
BANDCH = 4           # s-chunks feeding one t-subtile (covers +-133 band)
BANDW = 133          # mask support: |s - t| <= BANDW
W1PACK = 2           # w1 d-chunks packed per DMA row (3.5 KB descriptors)
W2PACK = 4           # w2T d-chunks packed per DMA row (4 KB descriptors)
XSPLIT = 4           # leading s-chunks per x group DMA (rest in 2nd DMA)

_NC = None


def _band(j):
    """Chunk-aligned t-column range [lo, hi) of attn chunk j phase B reads."""
    lo = P * max(0, j - (BANDCH - 1))
    hi = P * min(NQ - 1, j) + P
    return lo, hi


def _hull(j):
    """True mask-support t-range [lo, hi) of attn chunk j (|s-t| <= 133)."""
    lo = max(0, P * j - MARGIN - BANDW)        # 128j - 325
    hi = min(TB, P * j + P - MARGIN + BANDW)   # 128j + 69
    return lo, hi


def _build_module():
    from concourse import bacc, bass, mybir, tile
    from concourse.tile_rust import add_dep_helper

    f32 = mybir.dt.float32
    f16 = mybir.dt.float16
    PSUM = bass.MemorySpace.PSUM

    nc = bacc.Bacc("TRN2", target_bir_lowering=False, debug=False)
    xT_d = nc.declare_dram_parameter("xT_s", [P, NCH, N], f16, isOutput=False)
    w1_d = nc.declare_dram_parameter(
        "w1_s", [DCH // W1PACK, P, W1PACK * SW], f16, isOutput=False)
    w2T_d = nc.declare_dram_parameter(
        "w2T_s", [DCH // W2PACK, P, W2PACK * TB], f16, isOutput=False)
    mask_d = nc.declare_dram_parameter(
        "mask_s", [P, NCH * TB], f16, isOutput=False)
    b2_d = nc.declare_dram_parameter("b2c_s", [P, NQ], f32, isOutput=False)
    yT_d = nc.declare_dram_parameter("yT_s", [TB, N], f16, isOutput=True)

    with tile.TileContext(nc) as tc:
        with (
            tc.tile_pool(name="const", bufs=1) as cpool,
            tc.tile_pool(name="attn", bufs=1) as apool,
            tc.tile_pool(name="mp", bufs=1) as mp,
            tc.tile_pool(name="xp", bufs=NG) as xp,
            tc.tile_pool(name="yp", bufs=4) as yp,
        ):
            b2_t = cpool.tile([P, NQ], f32)
            nc.gpsimd.dma_start(b2_t[:], b2_d[:])
            m_t = mp.tile([P, NCH * TB], f16, name="m_t")
            nc.gpsimd.dma_start(m_t[:], mask_d[:])

            # attn SBUF tiles: zero the band-minus-hull gap regions once so
            # phase B reads exact zeros there (mask support ends at the hull)
            attn_sb = []
            for j in range(NCH):
                a_t = apool.tile([P, TB], f16, name=f"attn_sb{j}")
                blo, bhi = _band(j)
                hlo, hhi = _hull(j)
                if blo < hlo:
                    nc.vector.memset(a_t[:, blo:hlo], 0)
                if hhi < bhi:
                    nc.vector.memset(a_t[:, hhi:bhi], 0)
                attn_sb.append(a_t)

            # ---- Phase A: attn[s, t] = (w1.T @ w2T) * mask on the hull ----
            # weight DMAs alternate sync/scalar so each queue generates only
            # half the descriptors; arrival order tracks k-consumption order
            w_eng = [nc.sync, nc.scalar]
            w_idx = 0

            def w_dma(dst, src):
                nonlocal w_idx
                eng = w_eng[w_idx % 2]
                w_idx += 1
                return eng.dma_start(dst, src)

            w1_insts = []
            with (
                tc.tile_pool(name="w1p", bufs=8) as w1p,
                tc.tile_pool(name="w2p", bufs=4) as w2p,
                tc.tile_pool(name="psA", bufs=1, space=PSUM) as psA,
            ):
                attn_ps = [
                    psA.tile([P, TB], f32, name=f"attn_ps{j}") for j in range(NCH)
                ]
                kmm = {}
                for bb in range(DCH // W2PACK):
                    w2_t = w2p.tile([P, W2PACK * TB], f16)
                    if bb == 0:
                        # split so the k=0 matmuls can start sooner
                        nc.sync.dma_start(w2_t[:, :TB], w2T_d[0][:, :TB])
                        w_dma(w2_t[:, TB:], w2T_d[0][:, TB:])
                    else:
                        w_dma(w2_t[:], w2T_d[bb])
                    for hb in range(W2PACK // W1PACK):
                        pi = bb * (W2PACK // W1PACK) + hb
                        w1_t = w1p.tile([P, W1PACK * SW], f16)
                        if pi == 0:
                            nc.scalar.dma_start(w1_t[:, :SW], w1_d[0][:, :SW])
                            w1_insts.append(w_dma(w1_t[:, SW:], w1_d[0][:, SW:]))
                        else:
                            w1_insts.append(w_dma(w1_t[:], w1_d[pi]))
                        for half in range(W1PACK):
                            k = bb * W2PACK + hb * W1PACK + half
                            w1sl = w1_t[:, half * SW:(half + 1) * SW]
                            w2sl = w2_t[:, (hb * W1PACK + half) * TB:
                                        (hb * W1PACK + half + 1) * TB]
                            for j in (3, 2, 4, 1, 5, 0, 6):
                                lo, hi = _hull(j)
                                kmm[k] = nc.tensor.matmul(
                                    attn_ps[j][:, lo:hi],
                                    w1sl[:, j * P:(j + 1) * P],
                                    w2sl[:, lo:hi],
                                    start=(k == 0),
                                    stop=(k == DCH - 1),
                                )
                for j in range(NCH):
                    lo, hi = _hull(j)
                    nc.vector.tensor_mul(
                        attn_sb[j][:, lo:hi], attn_ps[j][:, lo:hi],
                        m_t[:, j * TB + lo:j * TB + hi]
                    )

            # ---- Phase B: yT[t, n] = gelu(attn.T @ xT + b2) on the band ----
            with tc.tile_pool(name="psB", bufs=4, space=PSUM) as psB:
                # x prefetch is gated on phase-A MATMUL progress (not on the
                # weight DMAs: a DMA-on-DMA dep shares queue semaphores and
                # stalls late phase-A ldweights).  Group 0 arrives in four
                # slices so phase B can start after only the first MB; later
                # groups follow ungated in SW-DGE queue order.
                x_t = []
                for g in range(NG):
                    xt = xp.tile([P, NCH * GN], f16, name="x_t", tag="x_t")
                    xt3 = xt.rearrange("p (j n) -> p j n", j=NCH)
                    if g == 0:
                        slices = (((0, XSPLIT), (0, GN // 2), 22),
                                  ((0, XSPLIT), (GN // 2, GN), 27),
                                  ((XSPLIT, NCH), (0, GN // 2), 30),
                                  ((XSPLIT, NCH), (GN // 2, GN), 31))
                    else:
                        slices = (((0, XSPLIT), (0, GN), None),
                                  ((XSPLIT, NCH), (0, GN), None))
                    for (jlo, jhi), (nlo, nhi), gate in slices:
                        xi = nc.gpsimd.dma_start(
                            xt3[:, jlo:jhi, nlo:nhi],
                            xT_d[:, jlo:jhi, g * GN + nlo:g * GN + nhi],
                        )
                        if gate is not None:
                            add_dep_helper(
                                xi.ins, kmm[gate].ins,
                                sync=True, reason="pace x behind phase A",
                            )
                    x_t.append(xt)
                for g in range(NG):
                    xt = x_t[g]
                    for q in range(NQ):
                        y_sb = yp.tile([P, GN], f16, name="y_sb", tag="y_sb")
                        last = (g == NG - 1 and q == NQ - 1)
                        for h in range(GN // (2 * MMN)):
                            y_ps = psB.tile([P, 2 * MMN], f32, name="y_ps",
                                            tag="y_ps")
                            for hh in range(2):
                                osl = slice(hh * MMN, (hh + 1) * MMN)
                                nlo = (2 * h + hh) * MMN
                                for c in range(BANDCH):
                                    j = q + c
                                    nc.tensor.matmul(
                                        y_ps[:, osl],
                                        attn_sb[j][:, q * P:(q + 1) * P],
                                        xt[:, j * GN + nlo:j * GN + nlo + MMN],
                                        start=(c == 0),
                                        stop=(c == BANDCH - 1),
                                    )
                            if last and h == GN // (2 * MMN) - 1:
                                # split the final gelu so the tail after the
                                # last matmul is as short as possible
                                for hh in range(2):
                                    nc.scalar.activation(
                                        y_sb[:, (2 * h + hh) * MMN:
                                             (2 * h + hh + 1) * MMN],
                                        y_ps[:, hh * MMN:(hh + 1) * MMN],
                                        mybir.ActivationFunctionType.Gelu,
                                        bias=b2_t[:, q:q + 1],
                                        scale=1.0,
                                    )
                            else:
                                nc.scalar.activation(
                                    y_sb[:, 2 * h * MMN:2 * (h + 1) * MMN],
                                    y_ps[:],
                                    mybir.ActivationFunctionType.Gelu,
                                    bias=b2_t[:, q:q + 1],
                                    scale=1.0,
                                )
                        if last:
                            # store the two halves in parallel on both queues
                            nc.scalar.dma_start(
                                yT_d[q * P:(q + 1) * P,
                                     g * GN:g * GN + GN // 2],
                                y_sb[:, :GN // 2])
                            nc.sync.dma_start(
                                yT_d[q * P:(q + 1) * P,
                                     g * GN + GN // 2:(g + 1) * GN],
                                y_sb[:, GN // 2:])
                        else:
                            st_eng = nc.scalar if (g + q) % 2 == 0 else nc.sync
                            st_eng.dma_start(
                                yT_d[q * P:(q + 1) * P, g * GN:(g + 1) * GN],
                                y_sb[:])

    nc.compile()
    nc.finalize()
    return nc


def _get_nc():
    global _NC
    if _NC is None:
        _NC = _build_module()
    return _NC


def prepare_in_maps(x, w1, w2, b2, sparse_mask):
    x = np.asarray(x, dtype=np.float32)
    w1 = np.asarray(w1, dtype=np.float32)
    w2 = np.asarray(w2, dtype=np.float32)
    b2 = np.asarray(b2, dtype=np.float32)
    sparse_mask = np.asarray(sparse_mask, dtype=np.float32)

    xT = np.ascontiguousarray(x.reshape(N, D).T.astype(np.float16))   # [s, n]
    w2T = np.ascontiguousarray(w2.T.astype(np.float16))               # [d, t]

    # Zero-pad the s axis by MARGIN on both sides so every core's window is
    # a plain slice; mask zeros make the padded rows contribute nothing.
    xT_pad = np.zeros((D + 2 * MARGIN, N), dtype=np.float16)
    xT_pad[MARGIN:MARGIN + D] = xT
    w1_pad = np.zeros((D, D + 2 * MARGIN), dtype=np.float16)
    w1_pad[:, MARGIN:MARGIN + D] = w1.astype(np.float16)
    mask_pad = np.zeros((D + 2 * MARGIN, D), dtype=np.float16)
    mask_pad[MARGIN:MARGIN + D] = sparse_mask.astype(np.float16)

    in_maps = []
    for i in range(NCORES):
        s0 = i * TB           # window start in padded coords
        t0 = i * TB
        w1win = w1_pad[:, s0:s0 + SW]                     # [D, SW]
        # pack W1PACK d-chunks per DMA row: [DCH/W1PACK, P, W1PACK*SW]
        w1_s = (w1win.reshape(DCH // W1PACK, W1PACK, P, SW)
                .transpose(0, 2, 1, 3)
                .reshape(DCH // W1PACK, P, W1PACK * SW))
        w2win = w2T[:, t0:t0 + TB]                        # [D, TB]
        w2_s = (w2win.reshape(DCH // W2PACK, W2PACK, P, TB)
                .transpose(0, 2, 1, 3)
                .reshape(DCH // W2PACK, P, W2PACK * TB))
        # x window and mask, partition-major: one DMA covers several
        # s-chunks with >= 4 KB descriptors
        xwin = (xT_pad[s0:s0 + SW].reshape(NCH, P, N)
                .transpose(1, 0, 2))                      # [P, NCH, N]
        mwin = (mask_pad[s0:s0 + SW, t0:t0 + TB].reshape(NCH, P, TB)
                .transpose(1, 0, 2)
                .reshape(P, NCH * TB))                    # [P, NCH*TB]
        in_maps.append({
            "xT_s": np.ascontiguousarray(xwin),
            "w1_s": np.ascontiguousarray(w1_s),
            "w2T_s": np.ascontiguousarray(w2_s),
            "mask_s": np.ascontiguousarray(mwin),
            "b2c_s": np.ascontiguousarray(b2[t0:t0 + TB].reshape(NQ, P).T),
        })
    return in_maps


def assemble(results):
    out = np.empty((N, D), dtype=np.float32)
    for i in range(NCORES):
        out[:, i * TB:(i + 1) * TB] = results[i]["yT_s"].T.astype(np.float32)
    return out.reshape(B, T, D)


def _band_ok(sparse_mask):
    """The Bass kernel only computes attn where each core's hull window
    covers the mask; verify every mask nonzero falls inside that region."""
    s_idx, t_idx = np.nonzero(np.asarray(sparse_mask) != 0)
    if len(s_idx) == 0:
        return True
    if np.abs(s_idx.astype(np.int64) - t_idx).max() > BANDW:
        return False
    w0 = (t_idx // TB) * TB - MARGIN          # per-core s-window start
    j = (s_idx - w0) // P                     # s-chunk within window
    q = (t_idx % TB) // P                     # t-subtile
    return bool(np.all((j >= q) & (j <= q + BANDCH - 1)
                       & (s_idx >= w0) & (s_idx < w0 + SW)))


def _reference_fallback(x, w1, w2, b2, sparse_mask):
    import jax
    import jax.numpy as jnp

    cpu = jax.devices("cpu")[0]
    with jax.default_device(cpu):
        attn = jnp.einsum("ds,td->st", jnp.asarray(w1), jnp.asarray(w2))
        attn = attn * jnp.asarray(sparse_mask)
        y = jnp.einsum("bds,st->bdt", jnp.asarray(x), attn) + jnp.asarray(b2)
        return np.asarray(jax.nn.gelu(y, approximate=False), dtype=np.float32)


def kernel(x, w1, w2, b2, sparse_mask):
    import time

    from concourse.bass_utils import run_bass_kernel_spmd

    if (np.shape(x) != (B, T, D) or np.shape(w1) != (D, D)
            or np.shape(w2) != (D, D) or np.shape(b2) != (D,)
            or np.shape(sparse_mask) != (D, D) or not _band_ok(sparse_mask)):
        return _reference_fallback(x, w1, w2, b2, sparse_mask)

    in_maps = prepare_in_maps(x, w1, w2, b2, sparse_mask)
    nc = _get_nc()
    last_err = None
    for attempt in range(3):
        try:
            res = run_bass_kernel_spmd(nc, in_maps, list(range(NCORES)))
            return assemble(res.results)
        except Exception as e:  # transient NRT/device errors: retry
            last_err = e
            time.sleep(2.0 * (attempt + 1))
    raise last_err


# revision 16
# speedup vs baseline: 1.1080x; 1.0508x over previous
"""Restore copy of the original staged baseline kernel (98820ns claim).

Diff vs kernel.py: original band (not hull) matmuls, w1 all on sync queue,
w2 on scalar, masks 7 DMAs on gpsimd, x 21 tile DMAs [NCH,P,N] layout on
gpsimd gated at w1 packs 11/13/15, y stores [P,1024] alternating.
"""

import numpy as np

B, T, D = 8, 768, 4096
N = B * T            # 6144 rows of x
NCORES = 8
TB = 512             # t-columns per core
P = 128
MARGIN = 192         # s-window extends this far before/after the t-block
SW = TB + 2 * MARGIN  # 896 s-window width
NCH = SW // P        # 7 s-chunks
DCH = D // P         # 32 d-chunks (contraction of phase A)
NQ = TB // P         # 4 t-subtiles per core
GN = 2048            # n-group width in phase B
NG = N // GN         # 3 n-groups
MMN = 512            # moving-operand / PSUM-bank free-dim cap per matmul
BANDCH = 4           # s-chunks feeding one t-subtile (covers +-133 band)
BANDW = 133          # mask support: |s - t| <= BANDW
W1PACK = 2           # w1 d-chunks packed per DMA row (3.5 KB descriptors)
W2PACK = 4           # w2T d-chunks packed per DMA row (4 KB descriptors)

_NC = None


def _band(j):
    lo = P * max(0, j - (BANDCH - 1))
    hi = P * min(NQ - 1, j) + P
    return lo, hi


def _hull(j):
    """True mask-support t-range [lo, hi) of attn chunk j (|s-t| <= 133)."""
    lo = max(0, P * j - MARGIN - BANDW)        # 128j - 325
    hi = min(TB, P * j + P - MARGIN + BANDW)   # 128j + 69
    return lo, hi


def _build_module():
    from concourse import bacc, bass, mybir, tile
    from concourse.tile_rust import add_dep_helper

    f32 = mybir.dt.float32
    f16 = mybir.dt.float16
    PSUM = bass.MemorySpace.PSUM

    nc = bacc.Bacc("TRN2", target_bir_lowering=False, debug=False)
    xT_d = nc.declare_dram_parameter("xT_s", [NCH, P, N], f16, isOutput=False)
    w1_d = nc.declare_dram_parameter(
        "w1_s", [DCH // W1PACK, P, W1PACK * SW], f16, isOutput=False)
    w2T_d = nc.declare_dram_parameter(
        "w2T_s", [DCH // W2PACK, P, W2PACK * TB], f16, isOutput=False)
    mask_d = nc.declare_dram_parameter("mask_s", [SW, TB], f16, isOutput=False)
    b2_d = nc.declare_dram_parameter("b2c_s", [P, NQ], f32, isOutput=False)
    yT_d = nc.declare_dram_parameter("yT_s", [TB, N], f16, isOutput=True)

    with tile.TileContext(nc) as tc:
        with (
            tc.tile_pool(name="const", bufs=1) as cpool,
            tc.tile_pool(name="attn", bufs=1) as apool,
            tc.tile_pool(name="mp", bufs=1) as mp,
            tc.tile_pool(name="xp", bufs=NG * NCH) as xp,
            tc.tile_pool(name="yp", bufs=6) as yp,
        ):
            b2_t = cpool.tile([P, NQ], f32)
            nc.gpsimd.dma_start(b2_t[:], b2_d[:])

            m_ts = []
            for j in range(NCH):
                m_t = mp.tile([P, TB], f16, name=f"m_t{j}")
                nc.gpsimd.dma_start(m_t[:], mask_d[j * P:(j + 1) * P, :])
                m_ts.append(m_t)

            # attn SBUF tiles: zero the band-minus-hull gap regions once
            # so phase B reads exact zeros there (mask support ends at hull)
            attn_sb = []
            for j in range(NCH):
                a_t = apool.tile([P, TB], f16, name=f"attn_sb{j}")
                blo, bhi = _band(j)
                hlo, hhi = _hull(j)
                if blo < hlo:
                    nc.vector.memset(a_t[:, blo:hlo], 0)
                if hhi < bhi:
                    nc.vector.memset(a_t[:, hhi:bhi], 0)
                attn_sb.append(a_t)
            w1_insts = []
            with (
                tc.tile_pool(name="w1p", bufs=8) as w1p,
                tc.tile_pool(name="w2p", bufs=4) as w2p,
                tc.tile_pool(name="psA", bufs=1, space=PSUM) as psA,
            ):
                attn_ps = [
                    psA.tile([P, TB], f32, name=f"attn_ps{j}") for j in range(NCH)
                ]
                for bb in range(DCH // W2PACK):
                    w2_t = w2p.tile([P, W2PACK * TB], f16)
                    nc.scalar.dma_start(w2_t[:], w2T_d[bb])
                    for hb in range(W2PACK // W1PACK):
                        pi = bb * (W2PACK // W1PACK) + hb
                        w1_t = w1p.tile([P, W1PACK * SW], f16)
                        w1_insts.append(nc.sync.dma_start(w1_t[:], w1_d[pi]))
                        for half in range(W1PACK):
                            k = bb * W2PACK + hb * W1PACK + half
                            w1sl = w1_t[:, half * SW:(half + 1) * SW]
                            w2sl = w2_t[:, (hb * W1PACK + half) * TB:
                                        (hb * W1PACK + half + 1) * TB]
                            for j in (3, 2, 4, 1, 5, 0, 6):
                                lo, hi = _hull(j)
                                nc.tensor.matmul(
                                    attn_ps[j][:, lo:hi],
                                    w1sl[:, j * P:(j + 1) * P],
                                    w2sl[:, lo:hi],
                                    start=(k == 0),
                                    stop=(k == DCH - 1),
                                )
                for j in range(NCH):
                    lo, hi = _hull(j)
                    nc.vector.tensor_mul(
                        attn_sb[j][:, lo:hi], attn_ps[j][:, lo:hi],
                        m_ts[j][:, lo:hi]
                    )

            with tc.tile_pool(name="psB", bufs=4, space=PSUM) as psB:
                for g in range(NG):
                    x_t = []
                    gate = {0: 11, 1: 13, 2: 15}[g]
                    for j in range(NCH):
                        xt = xp.tile([P, GN], f16, name="x_t", tag="x_t")
                        xi = nc.gpsimd.dma_start(
                            xt[:], xT_d[j, :, g * GN:(g + 1) * GN]
                        )
                        add_dep_helper(
                            xi.ins, w1_insts[gate].ins,
                            sync=True, reason="pace x prefetch behind w1",
                        )
                        x_t.append(xt)
                    for q in range(NQ):
                        for h in range(GN // (2 * MMN)):
                            y_ps = psB.tile([P, 2 * MMN], f32, name="y_ps",
                                            tag="y_ps")
                            for hh in range(2):
                                osl = slice(hh * MMN, (hh + 1) * MMN)
                                nsl = slice((2 * h + hh) * MMN,
                                            (2 * h + hh + 1) * MMN)
                                for c in range(BANDCH):
                                    j = q + c
                                    nc.tensor.matmul(
                                        y_ps[:, osl],
                                        attn_sb[j][:, q * P:(q + 1) * P],
                                        x_t[j][:, nsl],
                                        start=(c == 0),
                                        stop=(c == BANDCH - 1),
                                    )
                            y_sb = yp.tile([P, 2 * MMN], f16, name="y_sb",
                                           tag="y_sb")
                            nc.scalar.activation(
                                y_sb[:],
                                y_ps[:],
                                mybir.ActivationFunctionType.Gelu,
                                bias=b2_t[:, q:q + 1],
                                scale=1.0,
                            )
                            st_eng = nc.sync if (q + h) % 2 == 0 else nc.scalar
                            st_eng.dma_start(
                                yT_d[q * P:(q + 1) * P,
                                     g * GN + 2 * h * MMN:
                                     g * GN + 2 * (h + 1) * MMN],
                                y_sb[:],
                            )

    nc.compile()
    nc.finalize()
    return nc


def _get_nc():
    global _NC
    if _NC is None:
        _NC = _build_module()
    return _NC


def prepare_in_maps(x, w1, w2, b2, sparse_mask):
    x = np.asarray(x, dtype=np.float32)
    w1 = np.asarray(w1, dtype=np.float32)
    w2 = np.asarray(w2, dtype=np.float32)
    b2 = np.asarray(b2, dtype=np.float32)
    sparse_mask = np.asarray(sparse_mask, dtype=np.float32)

    xT = np.ascontiguousarray(x.reshape(N, D).T.astype(np.float16))   # [s, n]
    w2T = np.ascontiguousarray(w2.T.astype(np.float16))               # [d, t]

    xT_pad = np.zeros((D + 2 * MARGIN, N), dtype=np.float16)
    xT_pad[MARGIN:MARGIN + D] = xT
    w1_pad = np.zeros((D, D + 2 * MARGIN), dtype=np.float16)
    w1_pad[:, MARGIN:MARGIN + D] = w1.astype(np.float16)
    mask_pad = np.zeros((D + 2 * MARGIN, D), dtype=np.float16)
    mask_pad[MARGIN:MARGIN + D] = sparse_mask.astype(np.float16)

    in_maps = []
    for i in range(NCORES):
        s0 = i * TB
        t0 = i * TB
        w1win = w1_pad[:, s0:s0 + SW]
        w1_s = (w1win.reshape(DCH // W1PACK, W1PACK, P, SW)
                .transpose(0, 2, 1, 3)
                .reshape(DCH // W1PACK, P, W1PACK * SW))
        w2win = w2T[:, t0:t0 + TB]
        w2_s = (w2win.reshape(DCH // W2PACK, W2PACK, P, TB)
                .transpose(0, 2, 1, 3)
                .reshape(DCH // W2PACK, P, W2PACK * TB))
        in_maps.append({
            "xT_s": np.ascontiguousarray(
                xT_pad[s0:s0 + SW].reshape(NCH, P, N)),
            "w1_s": np.ascontiguousarray(w1_s),
            "w2T_s": np.ascontiguousarray(w2_s),
            "mask_s": np.ascontiguousarray(mask_pad[s0:s0 + SW, t0:t0 + TB]),
            "b2c_s": np.ascontiguousarray(b2[t0:t0 + TB].reshape(NQ, P).T),
        })
    return in_maps


def assemble(results):
    out = np.empty((N, D), dtype=np.float32)
    for i in range(NCORES):
        out[:, i * TB:(i + 1) * TB] = results[i]["yT_s"].T.astype(np.float32)
    return out.reshape(B, T, D)


def _band_ok(sparse_mask):
    s_idx, t_idx = np.nonzero(np.asarray(sparse_mask) != 0)
    if len(s_idx) == 0:
        return True
    if np.abs(s_idx.astype(np.int64) - t_idx).max() > BANDW:
        return False
    w0 = (t_idx // TB) * TB - MARGIN
    j = (s_idx - w0) // P
    q = (t_idx % TB) // P
    return bool(np.all((j >= q) & (j <= q + BANDCH - 1)
                       & (s_idx >= w0) & (s_idx < w0 + SW)))


def _reference_fallback(x, w1, w2, b2, sparse_mask):
    import jax
    import jax.numpy as jnp

    cpu = jax.devices("cpu")[0]
    with jax.default_device(cpu):
        attn = jnp.einsum("ds,td->st", jnp.asarray(w1), jnp.asarray(w2))
        attn = attn * jnp.asarray(sparse_mask)
        y = jnp.einsum("bds,st->bdt", jnp.asarray(x), attn) + jnp.asarray(b2)
        return np.asarray(jax.nn.gelu(y, approximate=False), dtype=np.float32)


def kernel(x, w1, w2, b2, sparse_mask):
    import time

    from concourse.bass_utils import run_bass_kernel_spmd

    if (np.shape(x) != (B, T, D) or np.shape(w1) != (D, D)
            or np.shape(w2) != (D, D) or np.shape(b2) != (D,)
            or np.shape(sparse_mask) != (D, D) or not _band_ok(sparse_mask)):
        return _reference_fallback(x, w1, w2, b2, sparse_mask)

    in_maps = prepare_in_maps(x, w1, w2, b2, sparse_mask)
    nc = _get_nc()
    last_err = None
    for attempt in range(3):
        try:
            res = run_bass_kernel_spmd(nc, in_maps, list(range(NCORES)))
            return assemble(res.results)
        except Exception as e:  # transient NRT/device errors: retry
            last_err = e
            time.sleep(2.0 * (attempt + 1))
    raise last_err
